# revision 1
# baseline (speedup 1.0000x reference)
"""MoE layer (cosine top-2 routing + per-expert FFN) on 8 Trainium2 cores.

Strategy (expert-parallel, two device phases):
  Phase A (gate NEFF, data-parallel): each core gates N/8 = 2048 tokens in
    fp32 (transpose -> x@Wp -> row-l2norm -> cosine logits -> top-2 + softmax).
  Host: builds per-expert compact dispatch lists from the 16 KB of routing
    metadata (the all-to-all of the sharding hint, done on host since the
    metadata is tiny).
  Phase B (FFN NEFF, expert-parallel): core e owns expert e. dma_gather
    collects its ~4K routed token rows, PE-transposes them to feature-major,
    then runs linear->GELU->linear with float32r (tf32-like) matmuls over H
    in four 1024-wide quarters, scaling by the gate, accumulating the
    feature-major output via DMA-accumulate. Host scatters compact outputs
    back and adds the residual x (top-2 softmax gates sum to 1 exactly).
"""
import sys
import numpy as np

sys.path.insert(0, "/opt/trn_rl_repo")

import concourse.bass as bass  # noqa: E402
import concourse.tile as tile  # noqa: E402
from concourse import bacc, mybir  # noqa: E402
from concourse import masks  # noqa: E402
from concourse.bass_utils import run_bass_kernel_spmd  # noqa: E402

f32 = mybir.dt.float32
f32r = mybir.dt.float32r
i16 = mybir.dt.int16
u32 = mybir.dt.uint32
AF = mybir.ActivationFunctionType
ALU = mybir.AluOpType

N, D, H, E = 16384, 1024, 4096, 8
PROJ = 256
NS = N // 8            # tokens per core in the gate phase
KC = D // 128          # 8 d-chunks
PC = PROJ // 128       # 2 proj-chunks
TB = 512               # token block
NTB = NS // TB         # 4 gate-phase blocks per core
C_PAD = 4608           # padded per-expert token capacity (max count 4254)
NB = C_PAD // TB       # 9 FFN blocks
NQ = 4                 # H quarters
HQ = H // NQ           # 1024
HC = HQ // 128         # 8 h-chunks per quarter
CLAMP_MAX = float(np.log(100.0))


def build_gate_nc(num_devices=8):
    nc = bacc.Bacc("TRN2", target_bir_lowering=False, debug=False,
                   enable_asserts=False, num_devices=num_devices)
    xs_ap = nc.dram_tensor("xs", [NS, D], f32, kind="ExternalInput").ap()
    wp_ap = nc.dram_tensor("wp", [D, PROJ], f32, kind="ExternalInput").ap()
    bp_ap = nc.dram_tensor("bp", [PROJ], f32, kind="ExternalInput").ap()
    simw_ap = nc.dram_tensor("simw", [PROJ, E], f32, kind="ExternalInput").ap()
    temp_ap = nc.dram_tensor("temp", [1], f32, kind="ExternalInput").ap()
    ti_ap = nc.dram_tensor("ti", [NS, 2], u32, kind="ExternalOutput").ap()
    tg_ap = nc.dram_tensor("tg", [NS, 2], f32, kind="ExternalOutput").ap()

    with tile.TileContext(nc) as tc:
        with (
            tc.tile_pool(name="const", bufs=1) as cpool,
            tc.tile_pool(name="io", bufs=2) as io,
            tc.tile_pool(name="work", bufs=2) as work,
            tc.tile_pool(name="small", bufs=2) as small,
            tc.tile_pool(name="ps_pp", bufs=2, space="PSUM") as ps_pp,
            tc.tile_pool(name="ps_mb", bufs=1, space="PSUM") as ps_mb,
            tc.tile_pool(name="ps_tp", bufs=3, space="PSUM") as ps_tp,
            tc.tile_pool(name="ps_sm", bufs=1, space="PSUM") as ps_sm,
            tc.tile_pool(name="ps_lg", bufs=1, space="PSUM") as ps_lg,
        ):
            ident = cpool.tile([128, 128], f32)
            masks.make_identity(nc, ident[:])
            ones = cpool.tile([128, 1], f32)
            nc.vector.memset(ones[:], 1.0)
            one_row = cpool.tile([1, 128], f32)
            nc.vector.memset(one_row[:], 1.0)

            wp = cpool.tile([128, KC, PROJ], f32)
            nc.sync.dma_start(wp[:], wp_ap.rearrange("(kc p) m -> p kc m", p=128))
            bp = cpool.tile([128, PC], f32)
            nc.sync.dma_start(bp[:], bp_ap.rearrange("(c p) -> p c", p=128))
            simw = cpool.tile([128, PC, E], f32)
            nc.sync.dma_start(simw[:], simw_ap.rearrange("(c p) e -> p c e", p=128))
            tempt = cpool.tile([1, 1], f32)
            nc.sync.dma_start(tempt[:], temp_ap.unsqueeze(0))

            scale_t = cpool.tile([1, 1], f32)
            nc.vector.tensor_scalar_min(scale_t[:], tempt[:], CLAMP_MAX)
            nc.scalar.activation(scale_t[:], scale_t[:], AF.Exp)

            # 1 / max(||sim[:, e]||, eps)
            sim_sq = small.tile([128, PC, E], f32)
            nc.vector.tensor_mul(sim_sq[:], simw[:], simw[:])
            sn_ps_t = ps_sm.tile([1, TB], f32, tag="sm")
            sn_ps = sn_ps_t[:, 0:E]
            for pc in range(PC):
                nc.tensor.matmul(sn_ps[:], ones[:], sim_sq[:, pc, :],
                                 start=(pc == 0), stop=(pc == PC - 1))
            sninv = cpool.tile([1, E], f32)
            nc.scalar.activation(sninv[:], sn_ps[:], AF.Sqrt)
            nc.vector.tensor_scalar_max(sninv[:], sninv[:], 1e-12)
            nc.vector.reciprocal(sninv[:], sninv[:])
            snb_ps_t = ps_mb.tile([128, TB], f32, tag="mb")
            snb_ps = snb_ps_t[:, 0:E]
            nc.tensor.matmul(snb_ps[:], one_row[:], sninv[:], start=True, stop=True)
            snb = cpool.tile([128, E], f32)
            nc.vector.tensor_copy(snb[:], snb_ps[:])

            for tb in range(NTB):
                gbuf = io.tile([128, 4, D], f32)
                nc.sync.dma_start(
                    gbuf[:], xs_ap[tb * TB:(tb + 1) * TB, :].rearrange(
                        "(c p) d -> p c d", p=128))
                xt = work.tile([128, KC, TB], f32)
                for c4 in range(4):
                    for dc in range(KC):
                        tp = ps_tp.tile([128, 128], f32)
                        nc.tensor.transpose(
                            tp[:], gbuf[:, c4, dc * 128:(dc + 1) * 128], ident[:])
                        nc.vector.tensor_copy(
                            xt[:, dc, c4 * 128:(c4 + 1) * 128], tp[:])
                projn = work.tile([128, PC, TB], f32)
                for pc in range(PC):
                    pp = ps_pp.tile([128, TB], f32)
                    for k in range(KC):
                        nc.tensor.matmul(pp[:], wp[:, k, pc * 128:(pc + 1) * 128],
                                         xt[:, k, :], start=(k == 0), stop=(k == KC - 1))
                    nc.vector.tensor_scalar_add(projn[:, pc, :], pp[:], bp[:, pc:pc + 1])
                sq = work.tile([128, PC, TB], f32)
                nc.vector.tensor_mul(sq[:], projn[:], projn[:])
                r2_ps = ps_sm.tile([1, TB], f32, tag="sm")
                for pc in range(PC):
                    nc.tensor.matmul(r2_ps[:], ones[:], sq[:, pc, :],
                                     start=(pc == 0), stop=(pc == PC - 1))
                minv = small.tile([1, TB], f32)
                nc.scalar.activation(minv[:], r2_ps[:], AF.Sqrt)
                nc.vector.tensor_scalar_max(minv[:], minv[:], 1e-12)
                nc.vector.reciprocal(minv[:], minv[:])
                nc.vector.tensor_scalar_mul(minv[:], minv[:], scale_t[:])
                mb_ps = ps_mb.tile([128, TB], f32, tag="mb")
                nc.tensor.matmul(mb_ps[:], one_row[:], minv[:], start=True, stop=True)
                mb = small.tile([128, TB], f32)
                nc.vector.tensor_copy(mb[:], mb_ps[:])
                for pc in range(PC):
                    nc.vector.tensor_mul(projn[:, pc, :], projn[:, pc, :], mb[:])
                for c4 in range(4):
                    lg_ps = ps_lg.tile([128, E], f32)
                    for pc in range(PC):
                        nc.tensor.matmul(
                            lg_ps[:], projn[:, pc, c4 * 128:(c4 + 1) * 128],
                            simw[:, pc, :], start=(pc == 0), stop=(pc == PC - 1))
                    lg = small.tile([128, E], f32)
                    nc.vector.tensor_mul(lg[:], lg_ps[:], snb[:])
                    mx = small.tile([128, 8], f32)
                    mi = small.tile([128, 8], u32)
                    nc.vector.max_with_indices(mx[:], mi[:], lg[:])
                    gpk = small.tile([128, 2], f32)
                    d12 = small.tile([128, 1], f32)
                    nc.vector.tensor_sub(d12[:], mx[:, 0:1], mx[:, 1:2])
                    nc.scalar.activation(gpk[:, 0:1], d12[:], AF.Sigmoid)
                    nc.vector.tensor_scalar(gpk[:, 1:2], gpk[:, 0:1], -1.0, 1.0,
                                            ALU.mult, ALU.add)
                    ipk = small.tile([128, 2], u32)
                    nc.vector.tensor_copy(ipk[:], mi[:, 0:2])
                    row0 = (tb * 4 + c4) * 128
                    nc.sync.dma_start(ti_ap[row0:row0 + 128, :], ipk[:])
                    nc.sync.dma_start(tg_ap[row0:row0 + 128, :], gpk[:])
    nc.compile()
    return nc


def build_ffn_nc(num_devices=8):
    nc = bacc.Bacc("TRN2", target_bir_lowering=False, debug=False,
                   enable_asserts=False, num_devices=num_devices)
    x_ap = nc.dram_tensor("x", [N, D], f32, kind="ExternalInput").ap()
    w1_ap = nc.dram_tensor("w1", [D, H], f32, kind="ExternalInput").ap()
    w2_ap = nc.dram_tensor("w2", [H, D], f32, kind="ExternalInput").ap()
    b1_ap = nc.dram_tensor("b1", [H], f32, kind="ExternalInput").ap()
    b2_ap = nc.dram_tensor("b2", [D], f32, kind="ExternalInput").ap()
    idx_ap = nc.dram_tensor("idxw", [128, C_PAD // 16], i16, kind="ExternalInput").ap()
    g_ap = nc.dram_tensor("gates", [1, C_PAD], f32, kind="ExternalInput").ap()
    out_ap = nc.dram_tensor("outT", [D, C_PAD], f32, kind="ExternalOutput").ap()

    with tile.TileContext(nc) as tc:
        with (
            tc.tile_pool(name="const", bufs=1) as cpool,
            tc.tile_pool(name="wpool", bufs=1) as wpool,
            tc.tile_pool(name="gath", bufs=2) as gath,
            tc.tile_pool(name="xtp", bufs=2) as xtp,
            tc.tile_pool(name="htp", bufs=2) as htp,
            tc.tile_pool(name="stgp", bufs=4) as stgp,
            tc.tile_pool(name="gbp", bufs=2) as gbp,
            tc.tile_pool(name="dram", bufs=1, space="DRAM") as dram,
            tc.tile_pool(name="ps_tp", bufs=3, space="PSUM") as ps_tp,
            tc.tile_pool(name="ps_h", bufs=2, space="PSUM") as ps_h,
            tc.tile_pool(name="ps_f", bufs=2, space="PSUM") as ps_f,
            tc.tile_pool(name="ps_gb", bufs=1, space="PSUM") as ps_gb,
        ):
            ident = cpool.tile([128, 128], f32)
            masks.make_identity(nc, ident[:])
            idxs = cpool.tile([128, C_PAD // 16], i16)
            nc.sync.dma_start(idxs[:], idx_ap[:])
            b1t = cpool.tile([128, H // 128], f32)
            nc.sync.dma_start(b1t[:], b1_ap.rearrange("(c p) -> p c", p=128))
            b2t = cpool.tile([128, KC], f32)
            nc.sync.dma_start(b2t[:], b2_ap.rearrange("(c p) -> p c", p=128))
            grow = cpool.tile([1, C_PAD], f32)
            nc.sync.dma_start(grow[:], g_ap[:])
            one_row = cpool.tile([1, 128], f32)
            nc.vector.memset(one_row[:], 1.0)

            xt_spill = dram.tile([NB, 128, KC * TB], f32r)

            # Phase 0: gather routed token rows, transpose to feature-major,
            # spill to DRAM (re-read once per H quarter).
            for b in range(NB):
                gbuf = gath.tile([128, 4, D], f32)
                nc.gpsimd.dma_gather(gbuf[:], x_ap[:], idxs[:, b * 32:(b + 1) * 32],
                                     num_idxs=TB, num_idxs_reg=TB, elem_size=D)
                xt = xtp.tile([128, KC, TB], f32r)
                for c4 in range(4):
                    for dc in range(KC):
                        tp = ps_tp.tile([128, 128], f32)
                        nc.tensor.transpose(
                            tp[:], gbuf[:, c4, dc * 128:(dc + 1) * 128], ident[:])
                        nc.vector.tensor_copy(
                            xt[:, dc, c4 * 128:(c4 + 1) * 128], tp[:])
                nc.sync.dma_start(xt_spill[b], xt[:].rearrange("p k t -> p (k t)"))

            # FFN over H quarters; out += W2q^T gelu(W1q^T xT + b1) * gate
            for q in range(NQ):
                w1q = wpool.tile([128, KC, HQ], f32r, tag="w1q")
                nc.gpsimd.dma_start(
                    w1q[:], w1_ap[:, q * HQ:(q + 1) * HQ].rearrange(
                        "(kc p) h -> p kc h", p=128))
                w2q = wpool.tile([128, HC, D], f32r, tag="w2q")
                nc.gpsimd.dma_start(
                    w2q[:], w2_ap[q * HQ:(q + 1) * HQ, :].rearrange(
                        "(hc p) d -> p hc d", p=128))
                for b in range(NB):
                    xt = xtp.tile([128, KC, TB], f32r)
                    nc.sync.dma_start(
                        xt[:].rearrange("p k t -> p (k t)"), xt_spill[b])
                    gb_ps = ps_gb.tile([128, TB], f32)
                    nc.tensor.matmul(gb_ps[:], one_row[:],
                                     grow[:, b * TB:(b + 1) * TB],
                                     start=True, stop=True)
                    gbc = gbp.tile([128, TB], f32)
                    nc.vector.tensor_copy(gbc[:], gb_ps[:])
                    ht = htp.tile([128, HC, TB], f32r)
                    for hc in range(HC):
                        pp = ps_h.tile([128, TB], f32)
                        for k in range(KC):
                            nc.tensor.matmul(pp[:], w1q[:, k, hc * 128:(hc + 1) * 128],
                                             xt[:, k, :], start=(k == 0),
                                             stop=(k == KC - 1))
                        nc.scalar.activation(ht[:, hc, :], pp[:], AF.Gelu,
                                             bias=b1t[:, q * HC + hc:q * HC + hc + 1])
                    for dc in range(KC):
                        pf = ps_f.tile([128, TB], f32)
                        for hc in range(HC):
                            nc.tensor.matmul(pf[:], w2q[:, hc, dc * 128:(dc + 1) * 128],
                                             ht[:, hc, :], start=(hc == 0),
                                             stop=(hc == HC - 1))
                        if q == 0:
                            nc.vector.tensor_scalar_add(pf[:], pf[:], b2t[:, dc:dc + 1])
                        stg = stgp.tile([128, TB], f32)
                        nc.vector.tensor_mul(stg[:], pf[:], gbc[:])
                        nc.gpsimd.dma_start(
                            out_ap[dc * 128:(dc + 1) * 128, b * TB:(b + 1) * TB],
                            stg[:], accum_op=ALU.add)
    nc.compile()
    return nc


def pack_indices(idx_list, gate_list):
    """Compact per-expert token list -> (wrapped int16 idx table, gate row)."""
    C = len(idx_list)
    assert C <= C_PAD, (C, C_PAD)
    ids = np.zeros(C_PAD, np.int16)
    gts = np.zeros(C_PAD, np.float32)
    ids[:C] = idx_list
    gts[:C] = gate_list
    idxw16 = np.zeros((16, C_PAD // 16), np.int16)
    for b in range(NB):
        blk = ids[b * TB:(b + 1) * TB]
        idxw16[:, b * 32:(b + 1) * 32] = blk.reshape(32, 16).T
    # replicate across the 8 Q7 cores (each reads its own 16-partition group)
    idxw = np.tile(idxw16, (8, 1))
    return idxw, gts.reshape(1, C_PAD)


_NC_CACHE = {}


def _get_ncs():
    if "gate" not in _NC_CACHE:
        _NC_CACHE["gate"] = build_gate_nc()
    if "ffn" not in _NC_CACHE:
        _NC_CACHE["ffn"] = build_ffn_nc()
    return _NC_CACHE["gate"], _NC_CACHE["ffn"]


def kernel(x, Wp, bp, sim, temp, W1, b1, W2, b2):
    x = np.ascontiguousarray(np.asarray(x, dtype=np.float32))
    Wp = np.ascontiguousarray(np.asarray(Wp, dtype=np.float32))
    bp = np.ascontiguousarray(np.asarray(bp, dtype=np.float32))
    sim = np.ascontiguousarray(np.asarray(sim, dtype=np.float32))
    temp = np.ascontiguousarray(np.asarray(temp, dtype=np.float32))
    W1 = np.ascontiguousarray(np.asarray(W1, dtype=np.float32))
    b1 = np.ascontiguousarray(np.asarray(b1, dtype=np.float32))
    W2 = np.ascontiguousarray(np.asarray(W2, dtype=np.float32))
    b2 = np.ascontiguousarray(np.asarray(b2, dtype=np.float32))

    nc_gate, nc_ffn = _get_ncs()

    # Phase A: gating, token-sharded
    in_maps = [{"xs": x[c * NS:(c + 1) * NS], "wp": Wp, "bp": bp,
                "simw": sim, "temp": temp} for c in range(8)]
    res_a = run_bass_kernel_spmd(nc_gate, in_maps, core_ids=list(range(8)))
    ti = np.concatenate([r["ti"] for r in res_a.results]).astype(np.int64)
    tg = np.concatenate([r["tg"] for r in res_a.results])

    # Host dispatch: build per-expert compact slot lists
    in_maps_b = []
    idx_per_core = []
    for e in range(E):
        m1 = ti[:, 0] == e
        m2 = ti[:, 1] == e
        sel = np.nonzero(m1 | m2)[0]
        g = np.where(m1[sel], tg[sel, 0], tg[sel, 1]).astype(np.float32)
        idx_per_core.append(sel)
        idxw, gts = pack_indices(sel, g)
        in_maps_b.append({"x": x, "w1": W1[e], "w2": W2[e], "b1": b1[e],
                          "b2": b2[e], "idxw": idxw, "gates": gts})

    # Phase B: expert-parallel FFN
    res_b = run_bass_kernel_spmd(nc_ffn, in_maps_b, core_ids=list(range(8)))

    # Host combine: out = x + sum_e scatter(gate * ffn_e)
    out = x.copy()
    for e in range(E):
        sel = idx_per_core[e]
        outT = res_b.results[e]["outT"]
        out[sel] += outT[:, :len(sel)].T
    return out


# revision 2
# speedup vs baseline: 1.0285x; 1.0285x over previous
"""MoE layer (cosine top-2 routing + per-expert FFN) on 8 Trainium2 cores.

Strategy (expert-parallel, two device phases):
  Phase A (gate NEFF, data-parallel): each core gates N/8 = 2048 tokens in
    fp32 (transpose -> x@Wp -> row-l2norm -> cosine logits -> top-2 + softmax).
  Host: builds per-expert compact dispatch lists from the 16 KB of routing
    metadata (the all-to-all of the sharding hint, done on host since the
    metadata is tiny).
  Phase B (FFN NEFF, expert-parallel): core e owns expert e. dma_gather
    collects its ~4K routed token rows, PE-transposes them to feature-major,
    then runs linear->GELU->linear with float32r (tf32-like) matmuls over H
    in four 1024-wide quarters, scaling by the gate, accumulating the
    feature-major output via DMA-accumulate. Host scatters compact outputs
    back and adds the residual x (top-2 softmax gates sum to 1 exactly).
"""
import sys
import numpy as np

sys.path.insert(0, "/opt/trn_rl_repo")

import concourse.bass as bass  # noqa: E402
import concourse.tile as tile  # noqa: E402
from concourse import bacc, mybir  # noqa: E402
from concourse import masks  # noqa: E402
from concourse.bass_utils import run_bass_kernel_spmd  # noqa: E402

f32 = mybir.dt.float32
f32r = mybir.dt.float32r
i16 = mybir.dt.int16
u32 = mybir.dt.uint32
AF = mybir.ActivationFunctionType
ALU = mybir.AluOpType

N, D, H, E = 16384, 1024, 4096, 8
PROJ = 256
NS = N // 8            # tokens per core in the gate phase
KC = D // 128          # 8 d-chunks
PC = PROJ // 128       # 2 proj-chunks
TB = 512               # token block
NTB = NS // TB         # 4 gate-phase blocks per core
C_PAD = 4608           # padded per-expert token capacity (max count 4254)
NB = C_PAD // TB       # 9 FFN blocks
NQ = 4                 # H quarters
HQ = H // NQ           # 1024
HC = HQ // 128         # 8 h-chunks per quarter
CLAMP_MAX = float(np.log(100.0))


def build_gate_nc(num_devices=8):
    nc = bacc.Bacc("TRN2", target_bir_lowering=False, debug=False,
                   enable_asserts=False, num_devices=num_devices)
    xs_ap = nc.dram_tensor("xs", [NS, D], f32, kind="ExternalInput").ap()
    wp_ap = nc.dram_tensor("wp", [D, PROJ], f32, kind="ExternalInput").ap()
    bp_ap = nc.dram_tensor("bp", [PROJ], f32, kind="ExternalInput").ap()
    simw_ap = nc.dram_tensor("simw", [PROJ, E], f32, kind="ExternalInput").ap()
    temp_ap = nc.dram_tensor("temp", [1], f32, kind="ExternalInput").ap()
    ti_ap = nc.dram_tensor("ti", [NS, 2], u32, kind="ExternalOutput").ap()
    tg_ap = nc.dram_tensor("tg", [NS, 2], f32, kind="ExternalOutput").ap()

    with tile.TileContext(nc) as tc:
        with (
            tc.tile_pool(name="const", bufs=1) as cpool,
            tc.tile_pool(name="io", bufs=2) as io,
            tc.tile_pool(name="work", bufs=2) as work,
            tc.tile_pool(name="small", bufs=2) as small,
            tc.tile_pool(name="ps_pp", bufs=2, space="PSUM") as ps_pp,
            tc.tile_pool(name="ps_mb", bufs=1, space="PSUM") as ps_mb,
            tc.tile_pool(name="ps_tp", bufs=3, space="PSUM") as ps_tp,
            tc.tile_pool(name="ps_sm", bufs=1, space="PSUM") as ps_sm,
            tc.tile_pool(name="ps_lg", bufs=1, space="PSUM") as ps_lg,
        ):
            ident = cpool.tile([128, 128], f32)
            masks.make_identity(nc, ident[:])
            ones = cpool.tile([128, 1], f32)
            nc.vector.memset(ones[:], 1.0)
            one_row = cpool.tile([1, 128], f32)
            nc.vector.memset(one_row[:], 1.0)

            wp = cpool.tile([128, KC, PROJ], f32)
            nc.sync.dma_start(wp[:], wp_ap.rearrange("(kc p) m -> p kc m", p=128))
            bp = cpool.tile([128, PC], f32)
            nc.sync.dma_start(bp[:], bp_ap.rearrange("(c p) -> p c", p=128))
            simw = cpool.tile([128, PC, E], f32)
            nc.sync.dma_start(simw[:], simw_ap.rearrange("(c p) e -> p c e", p=128))
            tempt = cpool.tile([1, 1], f32)
            nc.sync.dma_start(tempt[:], temp_ap.unsqueeze(0))

            scale_t = cpool.tile([1, 1], f32)
            nc.vector.tensor_scalar_min(scale_t[:], tempt[:], CLAMP_MAX)
            nc.scalar.activation(scale_t[:], scale_t[:], AF.Exp)

            # 1 / max(||sim[:, e]||, eps)
            sim_sq = small.tile([128, PC, E], f32)
            nc.vector.tensor_mul(sim_sq[:], simw[:], simw[:])
            sn_ps_t = ps_sm.tile([1, TB], f32, tag="sm")
            sn_ps = sn_ps_t[:, 0:E]
            for pc in range(PC):
                nc.tensor.matmul(sn_ps[:], ones[:], sim_sq[:, pc, :],
                                 start=(pc == 0), stop=(pc == PC - 1))
            sninv = cpool.tile([1, E], f32)
            nc.scalar.activation(sninv[:], sn_ps[:], AF.Sqrt)
            nc.vector.tensor_scalar_max(sninv[:], sninv[:], 1e-12)
            nc.vector.reciprocal(sninv[:], sninv[:])
            snb_ps_t = ps_mb.tile([128, TB], f32, tag="mb")
            snb_ps = snb_ps_t[:, 0:E]
            nc.tensor.matmul(snb_ps[:], one_row[:], sninv[:], start=True, stop=True)
            snb = cpool.tile([128, E], f32)
            nc.vector.tensor_copy(snb[:], snb_ps[:])

            for tb in range(NTB):
                gbuf = io.tile([128, 4, D], f32)
                nc.sync.dma_start(
                    gbuf[:], xs_ap[tb * TB:(tb + 1) * TB, :].rearrange(
                        "(c p) d -> p c d", p=128))
                xt = work.tile([128, KC, TB], f32)
                for c4 in range(4):
                    for dc in range(KC):
                        tp = ps_tp.tile([128, 128], f32)
                        nc.tensor.transpose(
                            tp[:], gbuf[:, c4, dc * 128:(dc + 1) * 128], ident[:])
                        nc.vector.tensor_copy(
                            xt[:, dc, c4 * 128:(c4 + 1) * 128], tp[:])
                projn = work.tile([128, PC, TB], f32)
                for pc in range(PC):
                    pp = ps_pp.tile([128, TB], f32)
                    for k in range(KC):
                        nc.tensor.matmul(pp[:], wp[:, k, pc * 128:(pc + 1) * 128],
                                         xt[:, k, :], start=(k == 0), stop=(k == KC - 1))
                    nc.vector.tensor_scalar_add(projn[:, pc, :], pp[:], bp[:, pc:pc + 1])
                sq = work.tile([128, PC, TB], f32)
                nc.vector.tensor_mul(sq[:], projn[:], projn[:])
                r2_ps = ps_sm.tile([1, TB], f32, tag="sm")
                for pc in range(PC):
                    nc.tensor.matmul(r2_ps[:], ones[:], sq[:, pc, :],
                                     start=(pc == 0), stop=(pc == PC - 1))
                minv = small.tile([1, TB], f32)
                nc.scalar.activation(minv[:], r2_ps[:], AF.Sqrt)
                nc.vector.tensor_scalar_max(minv[:], minv[:], 1e-12)
                nc.vector.reciprocal(minv[:], minv[:])
                nc.vector.tensor_scalar_mul(minv[:], minv[:], scale_t[:])
                mb_ps = ps_mb.tile([128, TB], f32, tag="mb")
                nc.tensor.matmul(mb_ps[:], one_row[:], minv[:], start=True, stop=True)
                mb = small.tile([128, TB], f32)
                nc.vector.tensor_copy(mb[:], mb_ps[:])
                for pc in range(PC):
                    nc.vector.tensor_mul(projn[:, pc, :], projn[:, pc, :], mb[:])
                for c4 in range(4):
                    lg_ps = ps_lg.tile([128, E], f32)
                    for pc in range(PC):
                        nc.tensor.matmul(
                            lg_ps[:], projn[:, pc, c4 * 128:(c4 + 1) * 128],
                            simw[:, pc, :], start=(pc == 0), stop=(pc == PC - 1))
                    lg = small.tile([128, E], f32)
                    nc.vector.tensor_mul(lg[:], lg_ps[:], snb[:])
                    mx = small.tile([128, 8], f32)
                    mi = small.tile([128, 8], u32)
                    nc.vector.max_with_indices(mx[:], mi[:], lg[:])
                    gpk = small.tile([128, 2], f32)
                    d12 = small.tile([128, 1], f32)
                    nc.vector.tensor_sub(d12[:], mx[:, 0:1], mx[:, 1:2])
                    nc.scalar.activation(gpk[:, 0:1], d12[:], AF.Sigmoid)
                    nc.vector.tensor_scalar(gpk[:, 1:2], gpk[:, 0:1], -1.0, 1.0,
                                            ALU.mult, ALU.add)
                    ipk = small.tile([128, 2], u32)
                    nc.vector.tensor_copy(ipk[:], mi[:, 0:2])
                    row0 = (tb * 4 + c4) * 128
                    nc.sync.dma_start(ti_ap[row0:row0 + 128, :], ipk[:])
                    nc.sync.dma_start(tg_ap[row0:row0 + 128, :], gpk[:])
    nc.compile()
    return nc


def build_ffn_nc(num_devices=8):
    nc = bacc.Bacc("TRN2", target_bir_lowering=False, debug=False,
                   enable_asserts=False, num_devices=num_devices)
    x_ap = nc.dram_tensor("x", [N, D], f32, kind="ExternalInput").ap()
    w1_ap = nc.dram_tensor("w1", [D, H], f32, kind="ExternalInput").ap()
    w2_ap = nc.dram_tensor("w2", [H, D], f32, kind="ExternalInput").ap()
    b1_ap = nc.dram_tensor("b1", [H], f32, kind="ExternalInput").ap()
    b2_ap = nc.dram_tensor("b2", [D], f32, kind="ExternalInput").ap()
    idx_ap = nc.dram_tensor("idxw", [128, C_PAD // 16], i16, kind="ExternalInput").ap()
    g_ap = nc.dram_tensor("gates", [1, C_PAD], f32, kind="ExternalInput").ap()
    out_ap = nc.dram_tensor("outT", [D, C_PAD], f32, kind="ExternalOutput").ap()
    TC = TB // 128
    IPB = TB // 16
    SG = 4                      # d-chunks per staged accumulate-DMA

    with tile.TileContext(nc) as tc:
        with (
            tc.tile_pool(name="const", bufs=1) as cpool,
            tc.tile_pool(name="w1p", bufs=1) as w1p,
            tc.tile_pool(name="w2p", bufs=1) as w2p,
            tc.tile_pool(name="gath", bufs=1) as gath,
            tc.tile_pool(name="xtp", bufs=2) as xtp,
            tc.tile_pool(name="htp", bufs=2) as htp,
            tc.tile_pool(name="stgp", bufs=2) as stgp,
            tc.tile_pool(name="gbp", bufs=2) as gbp,
            tc.tile_pool(name="dram", bufs=1, space="DRAM") as dram,
            tc.tile_pool(name="ps_tp", bufs=4, space="PSUM") as ps_tp,
            tc.tile_pool(name="ps_h", bufs=2, space="PSUM") as ps_h,
            tc.tile_pool(name="ps_f", bufs=2, space="PSUM") as ps_f,
        ):
            ident = cpool.tile([128, 128], f32)
            masks.make_identity(nc, ident[:])
            idxs = cpool.tile([128, C_PAD // 16], i16)
            nc.sync.dma_start(idxs[:], idx_ap[:])
            b1t = cpool.tile([128, H // 128], f32)
            nc.sync.dma_start(b1t[:], b1_ap.rearrange("(c p) -> p c", p=128))
            b2t = cpool.tile([128, KC], f32)
            nc.sync.dma_start(b2t[:], b2_ap.rearrange("(c p) -> p c", p=128))
            grow = cpool.tile([1, C_PAD], f32)
            nc.sync.dma_start(grow[:], g_ap[:])
            one_row = cpool.tile([1, 128], f32)
            nc.vector.memset(one_row[:], 1.0)

            xt_spill = dram.tile([NB, 128, KC * TB], f32r)

            def gather_transpose(b):
                # gather routed rows token-major, PE-transpose to feature-major
                gbuf = gath.tile([128, TC, D], f32)
                nc.gpsimd.dma_gather(gbuf[:], x_ap[:],
                                     idxs[:, b * IPB:(b + 1) * IPB],
                                     num_idxs=TB, num_idxs_reg=TB, elem_size=D)
                xt = xtp.tile([128, KC, TB], f32r)
                for c4 in range(TC):
                    for dc in range(KC):
                        tp = ps_tp.tile([128, TB], f32, tag="tp", name="tp")
                        nc.tensor.transpose(
                            tp[:, 0:128], gbuf[:, c4, dc * 128:(dc + 1) * 128],
                            ident[:])
                        nc.vector.tensor_copy(
                            xt[:, dc, c4 * 128:(c4 + 1) * 128], tp[:, 0:128])
                nc.sync.dma_start(xt_spill[b], xt[:].rearrange("p k t -> p (k t)"))
                return xt

            for q in range(NQ):
                w1q = w1p.tile([128, KC, HQ], f32r, tag="w1q")
                for k in range(KC):
                    nc.gpsimd.dma_start(
                        w1q[:, k, :], w1_ap[k * 128:(k + 1) * 128,
                                            q * HQ:(q + 1) * HQ])
                w2q = w2p.tile([128, HC, D], f32r, tag="w2q")
                for hc in range(HC):
                    nc.gpsimd.dma_start(
                        w2q[:, hc, :],
                        w2_ap[q * HQ + hc * 128:q * HQ + (hc + 1) * 128, :])
                for b in range(NB):
                    if q == 0:
                        xt = gather_transpose(b)
                    else:
                        xt = xtp.tile([128, KC, TB], f32r)
                        for k in range(KC):
                            nc.sync.dma_start(xt[:, k, :],
                                              xt_spill[b][:, k * TB:(k + 1) * TB])
                    gb_ps = ps_tp.tile([128, TB], f32, tag="tp", name="gb_ps")
                    nc.tensor.matmul(gb_ps[:], one_row[:],
                                     grow[:, b * TB:(b + 1) * TB],
                                     start=True, stop=True)
                    gbc = gbp.tile([128, TB], f32)
                    nc.vector.tensor_copy(gbc[:], gb_ps[:])
                    ht = htp.tile([128, HC, TB], f32r)
                    for hc in range(HC):
                        pp = ps_h.tile([128, TB], f32)
                        for k in range(KC):
                            nc.tensor.matmul(pp[:], w1q[:, k, hc * 128:(hc + 1) * 128],
                                             xt[:, k, :], start=(k == 0),
                                             stop=(k == KC - 1))
                        nc.scalar.activation(ht[:, hc, :], pp[:], AF.Gelu,
                                             bias=b1t[:, q * HC + hc:q * HC + hc + 1])
                    for dcg in range(KC // SG):
                        stg = stgp.tile([128, SG, TB], f32)
                        for dci in range(SG):
                            dc = dcg * SG + dci
                            pf = ps_f.tile([128, TB], f32)
                            for hc in range(HC):
                                nc.tensor.matmul(
                                    pf[:], w2q[:, hc, dc * 128:(dc + 1) * 128],
                                    ht[:, hc, :], start=(hc == 0),
                                    stop=(hc == HC - 1))
                            if q == 0:
                                nc.vector.tensor_scalar_add(pf[:], pf[:],
                                                            b2t[:, dc:dc + 1])
                            nc.vector.tensor_mul(stg[:, dci, :], pf[:], gbc[:])
                        nc.gpsimd.dma_start(
                            out_ap.rearrange("(dc p) c -> p dc c", p=128)[
                                :, dcg * SG:(dcg + 1) * SG, b * TB:(b + 1) * TB],
                            stg[:], accum_op=ALU.add)
    nc.compile()
    return nc


def pack_indices(idx_list, gate_list):
    """Compact per-expert token list -> (wrapped int16 idx table, gate row)."""
    C = len(idx_list)
    assert C <= C_PAD, (C, C_PAD)
    ids = np.zeros(C_PAD, np.int16)
    gts = np.zeros(C_PAD, np.float32)
    ids[:C] = idx_list
    gts[:C] = gate_list
    idxw16 = np.zeros((16, C_PAD // 16), np.int16)
    for b in range(NB):
        blk = ids[b * TB:(b + 1) * TB]
        idxw16[:, b * 32:(b + 1) * 32] = blk.reshape(32, 16).T
    # replicate across the 8 Q7 cores (each reads its own 16-partition group)
    idxw = np.tile(idxw16, (8, 1))
    return idxw, gts.reshape(1, C_PAD)


_NC_CACHE = {}


def _get_ncs():
    if "gate" not in _NC_CACHE:
        _NC_CACHE["gate"] = build_gate_nc()
    if "ffn" not in _NC_CACHE:
        _NC_CACHE["ffn"] = build_ffn_nc()
    return _NC_CACHE["gate"], _NC_CACHE["ffn"]


def kernel(x, Wp, bp, sim, temp, W1, b1, W2, b2):
    x = np.ascontiguousarray(np.asarray(x, dtype=np.float32))
    Wp = np.ascontiguousarray(np.asarray(Wp, dtype=np.float32))
    bp = np.ascontiguousarray(np.asarray(bp, dtype=np.float32))
    sim = np.ascontiguousarray(np.asarray(sim, dtype=np.float32))
    temp = np.ascontiguousarray(np.asarray(temp, dtype=np.float32))
    W1 = np.ascontiguousarray(np.asarray(W1, dtype=np.float32))
    b1 = np.ascontiguousarray(np.asarray(b1, dtype=np.float32))
    W2 = np.ascontiguousarray(np.asarray(W2, dtype=np.float32))
    b2 = np.ascontiguousarray(np.asarray(b2, dtype=np.float32))

    nc_gate, nc_ffn = _get_ncs()

    # Phase A: gating, token-sharded
    in_maps = [{"xs": x[c * NS:(c + 1) * NS], "wp": Wp, "bp": bp,
                "simw": sim, "temp": temp} for c in range(8)]
    res_a = run_bass_kernel_spmd(nc_gate, in_maps, core_ids=list(range(8)))
    ti = np.concatenate([r["ti"] for r in res_a.results]).astype(np.int64)
    tg = np.concatenate([r["tg"] for r in res_a.results])

    # Host dispatch: build per-expert compact slot lists
    in_maps_b = []
    idx_per_core = []
    for e in range(E):
        m1 = ti[:, 0] == e
        m2 = ti[:, 1] == e
        sel = np.nonzero(m1 | m2)[0]
        g = np.where(m1[sel], tg[sel, 0], tg[sel, 1]).astype(np.float32)
        idx_per_core.append(sel)
        idxw, gts = pack_indices(sel, g)
        in_maps_b.append({"x": x, "w1": W1[e], "w2": W2[e], "b1": b1[e],
                          "b2": b2[e], "idxw": idxw, "gates": gts})

    # Phase B: expert-parallel FFN
    res_b = run_bass_kernel_spmd(nc_ffn, in_maps_b, core_ids=list(range(8)))

    # Host combine: out = x + sum_e scatter(gate * ffn_e)
    out = x.copy()
    for e in range(E):
        sel = idx_per_core[e]
        outT = res_b.results[e]["outT"]
        out[sel] += outT[:, :len(sel)].T
    return out
